# revision 28
# baseline (speedup 1.0000x reference)
"""DogeDynamicMaskAttention Trainium2 kernel (v2: transposed-scores).

Sharding: 8 cores = 2 batches x 4 head-groups. Core c: batch b=c//4,
head-group g=c%4 -> heads [4g..4g+4), kv heads {2g, 2g+1}.

Device program (SPMD; identical program on all cores, different data):
  - x resident in SBUF as bf16 [128, 16cc x 2048tok]; all matmuls bf16
    (fp32 PSUM accumulate).
  - q/k projections into transposed layout [D, S] with RoPE
    (perm-matmul rotate-half + DVE combine); SCALING folded into Wq.
  - v projected directly into NATURAL layout [tok, D] (stationary = x
    chunk), with the dt columns (Wdt@Wv folded on host) appended to the
    same matmul -> dt needs no separate pass.
  - kthvalue threshold via 16-step bisection on bf16 bit space (dyn is
    quantized to bf16 for the count; near-threshold flips are within the
    rel-err budget). Penalized dyn row transposed to a column layout
    [128key, tile*4+h].
  - attention computed TRANSPOSED: sc[k, q] = (K^T Q), so the dynamic
    mask becomes a per-partition bias on the Exp activation (free), the
    P tiles come out of the exp already in the [key, query] layout that
    P@V needs (no PE transposes), l = column sums via ones-column
    matmuls, and 1/l is applied to the small output via a rank-1
    broadcast matmul + one DVE multiply.
  - causal masking by restricting each key-tile's query range; diagonal
    blocks get a DVE add of the host-transposed mask block.
  - fully-masked (degenerate) rows give l == 0; host detects via the l
    output (and any non-finite rows) and recomputes those rows in numpy.
"""
import sys
import numpy as np

sys.path.insert(0, "/root/.axon_site/_ro/trn_rl_repo")

import concourse.bass as bass  # noqa: E402,F401
from concourse import bacc  # noqa: E402
import concourse.tile as tile  # noqa: E402
import concourse.mybir as mybir  # noqa: E402
from concourse.bass_utils import run_bass_kernel_spmd  # noqa: E402
from concourse.alu_op_type import AluOpType  # noqa: E402
import ml_dtypes  # noqa: E402

F32 = mybir.dt.float32
F32R = mybir.dt.float32r
BF16 = mybir.dt.bfloat16
I32 = mybir.dt.int32
AF = mybir.ActivationFunctionType
AX = mybir.AxisListType.X
BF = ml_dtypes.bfloat16

B, S, HID = 2, 2048, 2048
H, KV, D = 16, 8, 128
HPC, KVPC = 4, 2
GROUPS = H // KV
NUM_DYN = S // 2
SCALING = D ** -0.5
MIN = float(np.finfo(np.float32).min)
BIG = 1.7e38
P = 128
NT = S // P          # 16
NQ = 4
QW = S // NQ         # 512
VW = KVPC * P + HPC  # 260: v cols + dt cols per cc chunk
NCORES = 8

_cache = {}


def _build_program(hostinfo):
    key = ("nc2", hostinfo)
    if key in _cache:
        return _cache[key]
    nc = bacc.Bacc("TRN2", target_bir_lowering=False, debug=False,
                   num_devices=NCORES)
    dram = {}
    for name, shape, dt in [
            ("xf", [P, NT * S], F32R),
            ("wqr", [P, NT * HPC * P], BF16),
            ("wkr", [P, NT * KVPC * P], BF16),
            ("wvr", [P, NT * KVPC * P], BF16),
            ("wdtr", [P, NT * HPC], F32R),
            ("wor", [P, NT * HPC * P], BF16),
            ("cosT", [P, S], BF16), ("sinT", [P, S], BF16),
            ("varblkT", [P, NT * P], F32),
            ("acol", [HPC, 1], F32),
            ("perm", [P, P], F32), ("eye128", [P, P], F32),
            ("eye4", [HPC, HPC], F32), ("eye64", [64, 64], F32),
            ("selT", [HPC, 64], F32), ("sel64", [64, HPC], F32),
            ("ones11", [1, 1], F32),
            ("onescol", [P, 1], F32), ("ones1", [1, P], F32)]:
        dram[name] = nc.dram_tensor(name, shape, dt, kind="ExternalInput").ap()
    outg_d = nc.dram_tensor("outg", [P, NQ * NT * QW], BF16,
                            kind="ExternalOutput").ap()
    l_d = nc.dram_tensor("l_out", [1, HPC * S], BF16,
                     kind="ExternalOutput").ap()

    with tile.TileContext(nc) as tc:
        _emit(nc, tc, dram, outg_d, l_d, hostinfo)
    nc.compile()
    _cache[key] = nc
    return nc


def _emit(nc, tc, dram, outg_d, l_d, hostinfo):
    from contextlib import ExitStack
    # hostinfo: per (grp, kt): (qlo_off, [(blk_off, slot), ...]) or None
    att_plan = hostinfo

    ctx = ExitStack()
    consts = ctx.enter_context(tc.tile_pool(name="consts", bufs=1))
    _deferred = []

    def cst_cast(name, shape, dt):
        t = consts.tile(shape, F32, name=f"c_{name}")
        nc.sync.dma_start(t[:], dram[name])
        r = consts.tile(shape, dt, name=f"cr_{name}")
        nc.scalar.copy(r[:], t[:])
        return r

    def cst(name, shape):
        t = consts.tile(shape, F32, name=f"c_{name}")
        nc.sync.dma_start(t[:], dram[name])
        return t

    cstv = {}

    def emit_consts():
        cstv["acol_t"] = consts.tile([HPC, 1], F32, name="c_acol")
        nc.sync.dma_start(cstv["acol_t"][:], dram["acol"])
        cstv["perm_r"] = cst_cast("perm", [P, P], F32R)
        cstv["eye4_f"] = cst("eye4", [HPC, HPC])
        cstv["onescol_b"] = cst_cast("onescol", [P, 1], BF16)
        cstv["ones1_r"] = cst_cast("ones1", [1, P], F32R)
    kthc = consts.tile([HPC, 1], F32, name="kthc")
    nc.vector.memset(kthc[:], float(NUM_DYN) - 0.5)

    # persistent activations
    act = ctx.enter_context(tc.tile_pool(name="act", bufs=1))
    qkro = [act.tile([P, S], BF16, name=f"qro{h}") for h in range(HPC)]
    kro = [act.tile([P, S], BF16, name=f"kro{i}") for i in range(KVPC)]
    vnat = act.tile([P, NT * KVPC * P], BF16, name="vnat")
    varblkT_t = act.tile([P, NT * P], F32, name="varblkT_t")
    _deferred.append((varblkT_t, "varblkT"))
    dyncol = act.tile([P, NT * HPC], F32, name="dyncol")
    lcat = act.tile([1, HPC * S], BF16, name="lcat")

    with ExitStack() as w1:
        xp = w1.enter_context(tc.tile_pool(name="xp", bufs=1))
        xres = xp.tile([P, NT * S], BF16, name="xres")
        wvd_t = xp.tile([P, NT * KVPC * P], BF16, name="wvd_t")
        dt_t = xp.tile([HPC, S], F32, name="dt_t")
        wq_t = xp.tile([P, NT * HPC * P], BF16, name="wq_t")
        wk_t = xp.tile([P, NT * KVPC * P], BF16, name="wk_t")
        # one proj-phase PSUM pool: 4x dt rows + 3 proj banks = 7 banks;
        # rope / stage-A tiles ride the same tag rings.
        mps = w1.enter_context(tc.tile_pool(name="mps", bufs=1, space="PSUM"))
        DTAG = [f"dt{i}" for i in range(NQ)]
        PTAG = ["pa", "pb", "pc"]

        # ---- stream x in f32r: dt matmuls, bf16 x copy, q0-2/sg0 proj ----
        with ExitStack() as wx:
            xfp = wx.enter_context(tc.tile_pool(name="xfp", bufs=3))
            wdp = wx.enter_context(tc.tile_pool(name="wdp", bufs=1))
            wdt_t = wdp.tile([P, NT * HPC], F32R, name="wdt_t")
            xf0 = xfp.tile([P, S], F32R, name="xf_c", tag="xf")
            nc.sync.dma_start(xf0[:], dram["xf"][:, 0:S])
            nc.sync.dma_start(wdt_t[:], dram["wdtr"])
            dtps = [mps.tile([HPC, QW], F32, name=f"dtps{sg}", tag=DTAG[sg])
                    for sg in range(NQ)]
            ps0 = [mps.tile([P, QW], F32, name=f"ps0{oi}", tag=PTAG[oi])
                   for oi in range(3)]
            for cc in range(NT):
                if cc == 0:
                    xf_c = xf0
                else:
                    xf_c = xfp.tile([P, S], F32R, name="xf_c", tag="xf")
                    nc.sync.dma_start(xf_c[:],
                                      dram["xf"][:, cc * S:(cc + 1) * S])
                nc.sync.dma_start(
                    wq_t[:, cc * HPC * P:(cc + 1) * HPC * P],
                    dram["wqr"][:, cc * HPC * P:(cc + 1) * HPC * P])
                for sg in range(NQ):
                    nc.tensor.matmul(
                        dtps[sg][:], wdt_t[:, cc * HPC:(cc + 1) * HPC],
                        xf_c[:, sg * QW:(sg + 1) * QW],
                        start=(cc == 0), stop=(cc == NT - 1),
                        skip_group_check=True)
                nc.scalar.copy(xres[:, cc * S:(cc + 1) * S], xf_c[:])
                for oi in range(3):
                    nc.tensor.matmul(
                        ps0[oi][:],
                        wq_t[:, cc * HPC * P + oi * P:
                             cc * HPC * P + (oi + 1) * P],
                        xres[:, cc * S: cc * S + QW],
                        start=(cc == 0), stop=(cc == NT - 1),
                        skip_group_check=True)
            for sg in range(NQ):
                nc.scalar.copy(dt_t[:, sg * QW:(sg + 1) * QW], dtps[sg][:])
        # consts + deferred input DMAs, ordered by first use
        emit_consts()
        nc.sync.dma_start(wk_t[:], dram["wkr"])
        for t, name in _deferred:
            nc.sync.dma_start(t[:], dram[name])
        nc.sync.dma_start(wvd_t[:], dram["wvr"])

        # ---------------- dyn + kth bisection -> dyncol ----------------
        # Pure-DVE 22-step value-space bisection on [4, S] rows: every op in
        # the serial chain stays on the Vector engine (no cross-engine
        # semaphore round trips), and the whole chain overlaps the proj
        # matmuls. Counts are exact f32 on the same data the penalty
        # compares, so the mask matches the threshold exactly.
        if True:
            dyq = w1.enter_context(tc.tile_pool(name="dyq", bufs=1))
            dps = w1.enter_context(
                tc.tile_pool(name="dps", bufs=1, space="PSUM"))
            nc.scalar.activation(dt_t[:], dt_t[:], AF.Exp)
            nc.scalar.activation(dt_t[:], dt_t[:], AF.Ln, bias=1.0)
            dyn_t = dyq.tile([HPC, S], F32, name="dyn_t")
            nc.scalar.activation(dyn_t[:], dt_t[:], AF.Exp,
                                 scale=cstv["acol_t"][:])
            hi = dyq.tile([HPC, 1], F32, name="hi")
            nc.vector.reduce_max(hi[:], dyn_t[:], axis=AX)
            nc.vector.tensor_scalar(hi[:], hi[:], 1.001, 0.01,
                                    op0=AluOpType.mult, op1=AluOpType.add)
            lo = dyq.tile([HPC, 1], F32, name="lo")
            nc.vector.memset(lo[:], 0.0)
            mid = dyq.tile([HPC, 1], F32, name="mid")
            scr = dyq.tile([HPC, S], F32, name="scr", tag="d4", bufs=2)
            cnt = dyq.tile([HPC, 1], F32, name="cnt")
            cges = dyq.tile([HPC, 1], I32, name="cges")
            cltv = dyq.tile([HPC, 1], I32, name="cltv")
            for _ in range(22):
                nc.vector.tensor_tensor(mid[:], hi[:], lo[:],
                                        op=AluOpType.add)
                nc.vector.tensor_scalar(mid[:], mid[:], 0.5, None,
                                        op0=AluOpType.mult)
                nc.vector.tensor_scalar(scr[:], dyn_t[:], mid[:, 0:1], 0.0,
                                        op0=AluOpType.is_lt,
                                        op1=AluOpType.add,
                                        accum_out=cnt[:])
                nc.vector.tensor_scalar(cges[:], kthc[:], cnt[:, 0:1], None,
                                        op0=AluOpType.is_lt)
                nc.vector.tensor_scalar(cltv[:], kthc[:], cnt[:, 0:1], None,
                                        op0=AluOpType.is_ge)
                nc.vector.copy_predicated(hi[:], cges[:], mid[:])
                nc.vector.copy_predicated(lo[:], cltv[:], mid[:])
            pen = dyq.tile([HPC, S], F32, name="pen", tag="d4", bufs=2)
            nc.vector.tensor_scalar(pen[:], dyn_t[:], lo[:, 0:1], -BIG,
                                    op0=AluOpType.is_lt, op1=AluOpType.mult)
            dynp = dyq.tile([HPC, S], F32, name="dynp", tag="d4", bufs=2)
            nc.vector.tensor_tensor(dynp[:], dyn_t[:], pen[:],
                                    op=AluOpType.add)
            dyc = dps.tile([P, NT * HPC], F32, name="dyc", tag="dp")
            for tt in range(NT):
                nc.tensor.transpose(
                    dyc[:, tt * HPC:(tt + 1) * HPC],
                    dynp[:, tt * P:(tt + 1) * P], cstv["eye4_f"][:])
            nc.scalar.copy(dyncol[:], dyc[:])

        # ---------------- q/k projections + RoPE ----------------
        with ExitStack() as w2:
            pjp = w2.enter_context(tc.tile_pool(name="pjp", bufs=3))
            csp = w2.enter_context(tc.tile_pool(name="csp", bufs=2))
            held = {(oi, 0): ps0[oi] for oi in range(3)}

            def emit_rope(sg, ps, cos_t, sin_t):
                for oi in range(6):
                    dst = qkro[oi] if oi < HPC else kro[oi - HPC]
                    f32t = pjp.tile([P, QW], F32R, name="pj32", tag="pj")
                    nc.scalar.copy(f32t[:], ps[oi][:])
                    qb = pjp.tile([P, QW], BF16, name="qb", tag="qb")
                    nc.scalar.copy(qb[:], ps[oi][:])
                    rh = mps.tile([P, QW], F32, name="rh", tag=DTAG[0])
                    nc.tensor.matmul(rh[:], cstv["perm_r"][:], f32t[:],
                                     start=True, stop=True,
                                     skip_group_check=True)
                    rb = pjp.tile([P, QW], BF16, name="rb", tag="qb")
                    nc.scalar.copy(rb[:], rh[:])
                    t1 = pjp.tile([P, QW], BF16, name="t1", tag="tb")
                    nc.gpsimd.tensor_tensor(t1[:], rb[:], sin_t[:],
                                            op=AluOpType.mult)
                    t2 = pjp.tile([P, QW], BF16, name="t2", tag="tb")
                    nc.gpsimd.tensor_tensor(t2[:], qb[:], cos_t[:],
                                            op=AluOpType.mult)
                    nc.gpsimd.tensor_tensor(
                        dst[:, sg * QW:(sg + 1) * QW], t1[:], t2[:],
                        op=AluOpType.add)

            pend = None
            for sg in range(NQ):
                cos_t = csp.tile([P, QW], BF16, name="cos_t", tag="cos")
                nc.sync.dma_start(cos_t[:],
                                  dram["cosT"][:, sg * QW:(sg + 1) * QW])
                sin_t = csp.tile([P, QW], BF16, name="sin_t", tag="sin")
                nc.sync.dma_start(sin_t[:],
                                  dram["sinT"][:, sg * QW:(sg + 1) * QW])
                tags = {0: PTAG[0], 1: PTAG[1], 2: PTAG[2],
                        3: DTAG[1], 4: DTAG[2], 5: DTAG[3]}
                ps = {}
                for oi in range(6):
                    if (oi, sg) in held:
                        ps[oi] = held.pop((oi, sg))
                        continue
                    ps[oi] = mps.tile([P, QW], F32, name=f"ps{oi}",
                                      tag=tags[oi])
                    for cc in range(NT):
                        xs = xres[:, cc * S + sg * QW: cc * S + (sg + 1) * QW]
                        if oi < HPC:
                            w_sl = wq_t[:, cc * HPC * P + oi * P:
                                        cc * HPC * P + (oi + 1) * P]
                        else:
                            i = oi - HPC
                            w_sl = wk_t[:, cc * KVPC * P + i * P:
                                        cc * KVPC * P + (i + 1) * P]
                        nc.tensor.matmul(ps[oi][:], w_sl, xs,
                                         start=(cc == 0), stop=(cc == NT - 1),
                                         skip_group_check=True)
                if pend is not None:
                    emit_rope(*pend)
                pend = (sg, ps, cos_t, sin_t)
            emit_rope(*pend)

        # ---------------- v natural ----------------
        VC = KVPC * P
        for tt in range(NT):
            av = mps.tile([P, VC], F32, name="av", tag=PTAG[tt % 2])
            for cc in range(NT):
                nc.tensor.matmul(
                    av[:], xres[:, cc * S + tt * P: cc * S + (tt + 1) * P],
                    wvd_t[:, cc * VC:(cc + 1) * VC],
                    start=(cc == 0), stop=(cc == NT - 1),
                    skip_group_check=True)
            nc.scalar.copy(vnat[:, tt * VC:(tt + 1) * VC], av[:])

    # ---------------- attention (transposed scores) ----------------
    with tc.tile_pool(name="expl", bufs=64) as expl, \
         tc.tile_pool(name="wop", bufs=1) as wop, \
         tc.tile_pool(name="attl", bufs=8) as attl, \
         tc.tile_pool(name="lvl", bufs=4) as lvl, \
         tc.tile_pool(name="otl", bufs=2) as otl, \
         tc.tile_pool(name="scp", bufs=2, space="PSUM") as scp, \
         tc.tile_pool(name="ovl", bufs=2, space="PSUM") as ovl, \
         tc.tile_pool(name="lpl", bufs=2, space="PSUM") as lpl, \
         tc.tile_pool(name="opl", bufs=1, space="PSUM") as opl, \
         tc.tile_pool(name="bcl", bufs=1, space="PSUM") as bcl:
        wor_t = wop.tile([P, NT * HPC * P], BF16, name="wor_t")
        nc.sync.dma_start(wor_t[:], dram["wor"])
        ats = {}

        def emit_qk(grp, hp):
            kv = hp
            hs = (2 * hp, 2 * hp + 1)
            kts = [kt for kt in range(NT) if att_plan[grp][kt] is not None]
            exps = {h: [] for h in hs}
            for kt in kts:
                qlo_off, vblks = att_plan[grp][kt]
                W = QW - qlo_off
                for h in hs:
                    sc = scp.tile([P, QW], F32, name="sc", tag="sc")
                    nc.tensor.matmul(
                        sc[:, :W], kro[kv][:, kt * P:(kt + 1) * P],
                        qkro[h][:, grp * QW + qlo_off:(grp + 1) * QW],
                        start=True, stop=True, skip_group_check=True)
                    for boff, slot in vblks:
                        nc.vector.tensor_tensor(
                            sc[:, boff:boff + P], sc[:, boff:boff + P],
                            varblkT_t[:, slot * P:(slot + 1) * P],
                            op=AluOpType.add)
                    e = expl.tile([P, QW], BF16, name="e", tag="e")
                    nc.scalar.activation(
                        e[:, qlo_off:], sc[:, :W], AF.Exp,
                        bias=dyncol[:, kt * HPC + h: kt * HPC + h + 1])
                    exps[h].append((e, qlo_off))
            return kts, exps

        def emit_lpv_pair(grp, hp, kts, expd):
            for h in (2 * hp, 2 * hp + 1):
                emit_lpv(grp, h, hp, kts, expd[h])

        def emit_lpv(grp, h, kv, kts, exps):
            lp = lpl.tile([1, QW], F32, name="lp", tag="lp")
            for i, (e, off) in enumerate(exps):
                nc.tensor.matmul(lp[:, off:], cstv["onescol_b"][:],
                                 e[:, off:],
                                 start=(i == 0), stop=(i == len(exps) - 1),
                                 skip_group_check=True)
            lrow_r = lvl.tile([1, QW], F32R, name="lrow_r", tag="lr")
            with nc.allow_low_precision(reason="l broadcast for 1/l"):
                nc.scalar.copy(lrow_r[:], lp[:])
            nc.scalar.copy(
                lcat[:, h * S + grp * QW: h * S + (grp + 1) * QW], lp[:])
            ov = ovl.tile([P, QW], F32, name="ov", tag="ov")
            for i, (e, off) in enumerate(exps):
                kt = kts[i]
                nc.tensor.matmul(
                    ov[:, off:],
                    vnat[:, kt * KVPC * P + kv * P:
                         kt * KVPC * P + (kv + 1) * P],
                    e[:, off:],
                    start=(i == 0), stop=(i == len(exps) - 1),
                    skip_group_check=True)
            bc = bcl.tile([P, QW], F32, name="bc", tag="bc")
            nc.tensor.matmul(bc[:], cstv["ones1_r"][:], lrow_r[:],
                             start=True, stop=True, skip_group_check=True)
            bcs = lvl.tile([P, QW], F32, name="bcs", tag="bcs")
            nc.vector.reciprocal_approx_fast(bcs[:], bc[:])
            at = attl.tile([P, QW], BF16, name="at", tag="at")
            nc.vector.tensor_tensor(at[:], ov[:], bcs[:],
                                    op=AluOpType.mult)
            ats[(grp, h)] = at

        def emit_outproj(grp):
            ot = otl.tile([P, NT * QW], BF16, name="ot", tag="ot")
            for ht in range(NT):
                op = opl.tile([P, QW], F32, name="op", tag="op")
                for h in range(HPC):
                    nc.tensor.matmul(
                        op[:], wor_t[:, (ht * HPC + h) * P:
                                     (ht * HPC + h + 1) * P],
                        ats[(grp, h)][:], start=(h == 0), stop=(h == HPC - 1),
                        skip_group_check=True)
                nc.vector.tensor_copy(ot[:, ht * QW:(ht + 1) * QW], op[:])
                if ht % 4 == 3:
                    o0 = (ht - 3) * QW
                    nc.sync.dma_start(
                        outg_d[:, grp * NT * QW + o0:
                               grp * NT * QW + o0 + 4 * QW],
                        ot[:, o0:o0 + 4 * QW])

        # software pipeline over head-pairs (shared kv => shared qk/pv
        # stationary loads): qk of pair i+1 is emitted before l/pv of
        # pair i so the Exp latency hides under other PE work.
        items = [(grp, hp) for grp in range(NQ) for hp in range(KVPC)]
        pending = []
        for it in items:
            state = emit_qk(*it)
            pending.append((it, state))
            if len(pending) > 1:
                pit, pstate = pending.pop(0)
                emit_lpv_pair(pit[0], pit[1], *pstate)
                if pit[1] == KVPC - 1:
                    emit_outproj(pit[0])
        for pit, pstate in pending:
            emit_lpv_pair(pit[0], pit[1], *pstate)
            if pit[1] == KVPC - 1:
                emit_outproj(pit[0])
    nc.sync.dma_start(l_d, lcat[:])
    ctx.close()


def _pack16(a):
    """[X*128, F] -> [128, X*F] (chunk-major rearrange), contiguous."""
    X = a.shape[0] // P
    return np.ascontiguousarray(
        a.reshape(X, P, -1).transpose(1, 0, 2).reshape(P, -1))


def _host_prep(hidden_states, cos, sin, attention_mask, Wq, Wk, Wv, A, Wdt,
               Wo):
    perm = np.zeros((P, P), dtype=np.float32)
    for j in range(64):
        perm[j + 64, j] = -1.0
        perm[j, j + 64] = 1.0
    eye128 = np.eye(P, dtype=np.float32)
    eye4 = np.eye(HPC, dtype=np.float32)
    eye64 = np.eye(64, dtype=np.float32)
    onescol = np.ones((P, 1), dtype=np.float32)
    ones1 = np.ones((1, P), dtype=np.float32)
    ones11 = np.ones((1, 1), dtype=np.float32)
    selT = np.zeros((HPC, 64), dtype=np.float32)
    sel64 = np.zeros((64, HPC), dtype=np.float32)
    for p in range(64):
        selT[p % HPC, p] = 1.0
        sel64[p, p % HPC] = 1.0

    in_maps = []
    plans = []
    for c in range(NCORES):
        b, g = divmod(c, 4)
        heads = list(range(4 * g, 4 * g + 4))
        xT = np.ascontiguousarray(hidden_states[b].T)           # [HID, S]
        xf = _pack16(xT).astype(np.float32)                     # [128, 16*S]
        wqT = (Wq[4 * g * D:(4 * g + 4) * D]
               * np.float32(SCALING)).T.astype(BF)              # [HID, 512]
        wqr = _pack16(wqT)
        wkT = Wk[2 * g * D:(2 * g + 2) * D].T.astype(BF)        # [HID, 256]
        wkr = _pack16(wkT)
        wvT = Wv[2 * g * D:(2 * g + 2) * D].T.astype(BF)        # [HID, 256]
        wvr = _pack16(wvT)
        wdtvT = (Wdt[heads].astype(np.float64)
                 @ Wv.astype(np.float64)).T.astype(np.float32)  # [HID, 4]
        wdtr = _pack16(wdtvT)
        woT = Wo[:, 4 * g * D:(4 * g + 4) * D].T                # [512, HID]
        # wor[p, (ht*4+h)*128+j] = woT[h*128+p, ht*128+j]
        wor = np.ascontiguousarray(
            woT.reshape(HPC, P, NT, P).transpose(1, 2, 0, 3)
            .reshape(P, NT * HPC * P)).astype(BF)
        acol = A[heads].astype(np.float32).reshape(HPC, 1)
        cosT = np.ascontiguousarray(cos[b].T).astype(BF)
        sinT = np.ascontiguousarray(sin[b].T).astype(BF)

        m = attention_mask[b, 0]
        mb = m.reshape(NT, P, NT, P)
        blk = np.empty((NT, NT), dtype=object)
        varlist = []
        for qt in range(NT):
            for kt in range(NT):
                blkv = mb[qt, :, kt, :]
                if np.all(blkv == 0):
                    blk[qt, kt] = ("Z", None)
                elif np.all(blkv <= -1e30):
                    blk[qt, kt] = ("M", None)
                else:
                    blk[qt, kt] = ("V", len(varlist))
                    varlist.append(np.maximum(blkv, -BIG).T)  # transposed
        # attention plan per (grp, kt): (qlo_off, [(blk_off, slot)...])
        plan = []
        for grp in range(NQ):
            qts = range(grp * 4, grp * 4 + 4)
            row = []
            for kt in range(NT):
                states = [blk[qt, kt][0] for qt in qts]
                if all(s == "M" for s in states):
                    row.append(None)
                    continue
                first = min(i for i, s in enumerate(states) if s != "M")
                # interior fully-masked blocks -> promote to -BIG V block
                for i in range(first + 1, 4):
                    if states[i] == "M":
                        blk[grp * 4 + i, kt] = ("V", len(varlist))
                        varlist.append(np.full((P, P), -BIG, np.float32))
                if kt == 0 and first != 0:
                    raise NotImplementedError("first key tile must cover "
                                              "the full query window")
                qlo_off = first * P
                vblks = []
                for i in range(first, 4):
                    st, slot = blk[grp * 4 + i, kt]
                    if st == "V":
                        vblks.append((i * P - qlo_off, slot))
                row.append((qlo_off, tuple(vblks)))
            if row[0] is None:
                raise NotImplementedError("key tile 0 fully masked")
            plan.append(tuple(row))
        if len(varlist) > NT:
            raise NotImplementedError("too many varying mask blocks")
        varblkT = np.zeros((P, NT * P), dtype=np.float32)
        for vi, blkv in enumerate(varlist):
            varblkT[:, vi * P:(vi + 1) * P] = blkv
        plans.append(tuple(plan))
        in_maps.append({
            "xf": xf, "wqr": wqr, "wkr": wkr, "wvr": wvr, "wdtr": wdtr,
            "wor": wor, "cosT": cosT, "sinT": sinT, "varblkT": varblkT,
            "acol": acol, "perm": perm, "eye128": eye128, "eye4": eye4,
            "eye64": eye64, "selT": selT, "sel64": sel64, "ones11": ones11,
            "onescol": onescol, "ones1": ones1,
        })
    if len(set(plans)) != 1:
        raise NotImplementedError("mask structure differs across cores")
    return in_maps, plans[0]


def _softplus64(x):
    x = x.astype(np.float64)
    return np.log1p(np.exp(-np.abs(x))) + np.maximum(x, 0)


def _repair_rows(out, bad, inputs):
    """Recompute rows flagged bad [B, S] with faithful numpy reference math."""
    if not bad.any():
        return out
    hs = inputs["hidden_states"]; cos = inputs["cos"]; sin = inputs["sin"]
    am = inputs["attention_mask"]; Wq = inputs["Wq"]; Wk = inputs["Wk"]
    Wv = inputs["Wv"]; A = inputs["A"]; Wdt = inputs["Wdt"]; Wo = inputs["Wo"]

    def rope(x, c, s):
        x1, x2 = x[..., :D // 2], x[..., D // 2:]
        return x * c + np.concatenate([-x2, x1], axis=-1) * s

    for b in range(B):
        rows = np.where(bad[b])[0]
        if len(rows) == 0:
            continue
        x = hs[b].astype(np.float32)
        k = (x @ Wk.T).reshape(S, KV, D)
        v = (x @ Wv.T).reshape(S, KV, D)
        k = rope(k, cos[b][:, None, :], sin[b][:, None, :])
        v_flat = v.reshape(S, KV * D)
        dt = v_flat @ Wdt.T
        dyn = np.exp(A[None, :] * _softplus64(dt)).astype(np.float32).T
        kth = np.sort(dyn, axis=-1)[:, NUM_DYN - 1:NUM_DYN]
        dmask = np.where(dyn < kth, MIN, dyn).astype(np.float32)
        for s_i in rows:
            q_row = (x[s_i] @ Wq.T).reshape(H, D)
            q_row = rope(q_row, cos[b][s_i][None, :], sin[b][s_i][None, :])
            attn_row = np.zeros((H, D), dtype=np.float32)
            for h in range(H):
                kvh = h // GROUPS
                sc = ((q_row[h] @ k[:, kvh].T) * np.float32(SCALING)
                      + np.maximum(dmask[h] + am[b, 0, s_i], MIN))
                w = np.exp(sc - sc.max())
                w = (w / w.sum()).astype(np.float32)
                attn_row[h] = w @ v[:, kvh]
            out[b, s_i] = attn_row.reshape(H * D) @ Wo.T
    return out


def kernel(**inputs):
    inputs = {k: np.asarray(v) for k, v in inputs.items()}
    in_maps, plan = _host_prep(**inputs)
    nc = _build_program(plan)
    res = run_bass_kernel_spmd(nc, in_maps, list(range(NCORES)))
    out = np.zeros((B, S, HID), dtype=np.float32)
    bad = np.zeros((B, S), dtype=bool)
    for c in range(NCORES):
        b = c // 4
        og = np.asarray(res.results[c]["outg"]).astype(np.float32)
        # og[p, ((grp*16)+ht)*512 + t] = outT[ht*128+p, grp*512+t]
        og = og.reshape(P, NQ, NT, QW).transpose(2, 0, 1, 3).reshape(HID, S)
        out[b] += og.T
        lv = np.asarray(res.results[c]["l_out"]).reshape(HPC, S)
        bad[b] |= (lv == 0).any(axis=0)
    bad |= ~np.isfinite(out).all(axis=2)
    out = _repair_rows(out, bad, inputs)
    return out
